# revision 3
# baseline (speedup 1.0000x reference)
"""Trainium2 Bass kernel: 2-layer LSTM (H=64, D=8, T=256) + FC head, batch 8192.

Strategy (pure data parallel, 8 cores x 1024 batch):
  - Stage s computes layer0 timestep s and layer1 timestep s-1 simultaneously,
    with all per-gate tensors stacked [layer0(64p); layer1(64p)] on 128
    partitions.  The stacked hidden state h = [h0_s; h1_{s-1}] is exactly the
    rhs the layer1 matmul of the next stage needs (K=128).
  - Per unit one 4-bank PSUM tile holds gates i|f|o|g.  ACT runs 3
    instructions per unit: one Sigmoid over [128,1536] (i,f,o across 3 banks),
    Tanh(g), Tanh(c).  ALL gate biases are folded into the matmuls via a
    persistent ones-row in the xh rhs: the layer0 matmul is widened to M=128 so
    its ones-row weight row broadcasts the layer1 biases into rows 64:128,
    which the layer1 matmul then accumulates onto (start=False).
  - Batch split into 2 subtiles of 512 that pipeline through the engines.
  - LSTM state (h, c, xh) lives in persistent ping-pong SBUF tensors (parity =
    stage index & 1); the h0->xh copy runs on the otherwise-idle GPSIMD.
  - x is transposed upfront to [t%16*8+d (partitions), tc*BC+b] bf16 layout via
    PE-transpose; staging double-buffered, loads split across the SP/ACT DMA
    queues, PSUM->SBUF copies split across DVE and ACT.
  - Stage 0 uses weights with zeroed layer1 biases so the layer1 half computes
    exactly 0 (matching h1_{-1} = 0); stage T computes a garbage-but-finite
    layer0 step while layer1 finishes h1_{T-1} for the FC head.
  - The 257 stages are split across two sequential TileContexts (event-sem
    values saturate in a single fully-unrolled context).
"""

import numpy as np
import ml_dtypes

import concourse.bass as bass
import concourse.bacc as bacc
import concourse.mybir as mybir
import concourse.tile as tile
from concourse.bass_utils import run_bass_kernel_spmd

F32 = mybir.dt.float32
BF16 = mybir.dt.bfloat16
AF = mybir.ActivationFunctionType

H = 64
D = 8
T_FULL = 256
B_TOTAL = 8192
N_CORES = 8
BC = B_TOTAL // N_CORES  # 1024 per core
NSUB = 2
BSUB = BC // NSUB  # 512

# my gate order in PSUM cols: i, f, o, g; value = reference gate row block
GATE_SLICES = [0, 1, 3, 2]


def _emit_transpose_phase(nc, tc, x_d, ident_sb, xT, t_steps):
    """Load x [BC, T*D] f32 from DRAM, transpose to xT [128(t%16*8+d), tc*BC+b] bf16."""
    n_tc = t_steps // 16
    with tc.sbuf_pool(name="xstage", bufs=2) as stpool, tc.psum_pool(
        name="ptp", bufs=2
    ) as ptpool:
        for half in range(2):  # b-chunks 0..3 then 4..7
            xs = []
            for i in range(4):
                bch = half * 4 + i
                xsi = stpool.tile([128, t_steps * D], F32, name=f"xs{i}", tag=f"xs{i}")
                eng = nc.sync if i % 2 == 0 else nc.scalar
                eng.dma_start(xsi, x_d[bch * 128 : (bch + 1) * 128, 0 : t_steps * D])
                xs.append(xsi)
            for tci in range(n_tc):
                pt = ptpool.tile([128, 512], F32, name="pt", tag="pt")
                for i in range(4):
                    nc.tensor.transpose(
                        pt[:, i * 128 : (i + 1) * 128],
                        xs[i][:, tci * 128 : (tci + 1) * 128],
                        ident_sb,
                    )
                dst = xT[:, tci * BC + half * 512 : tci * BC + (half + 1) * 512]
                if tci % 2 == 0:
                    nc.vector.tensor_copy(dst, pt)
                else:
                    nc.scalar.copy(dst, pt)
    # zero pad chunk for t == t_steps (stage t_steps reads it; garbage-but-finite path)
    nc.gpsimd.memset(xT[:, n_tc * BC : (n_tc + 1) * BC], 0.0)


class _Consts:
    pass


def _emit_stage(nc, spool, gpool, cst, s, u, n_stage):
    """Emit one (stage, subtile) unit."""
    rp = s % 2  # read parity
    wp = (s + 1) % 2  # write parity
    xh_r, h_r, c_r = cst.xh[u][rp], cst.h[u][rp], cst.c[u][rp]
    xh_w, h_w, c_w = cst.xh[u][wp], cst.h[u][wp], cst.c[u][wp]
    w0 = cst.w0s0 if s == 0 else cst.w0big

    P = gpool.tile([128, 2048], F32, name=f"P_u{u}", tag=f"P_u{u}")
    for j in range(4):  # gates i, f, o, g
        nc.tensor.matmul(
            P[:, j * 512 : (j + 1) * 512],
            w0[:, j * 128 : (j + 1) * 128],
            xh_r,
            start=True,
            stop=False,
        )
        nc.tensor.matmul(
            P[64:128, j * 512 : (j + 1) * 512],
            cst.w1[:, j * 64 : (j + 1) * 64],
            h_r,
            start=False,
            stop=True,
        )

    S_ifo = spool.tile([128, 1536], BF16, name=f"S_ifo_u{u}", tag=f"S_ifo_u{u}")
    nc.scalar.activation(S_ifo, P[:, 0:1536], AF.Sigmoid)
    T_g = spool.tile([128, 512], BF16, name=f"T_g_u{u}", tag=f"T_g_u{u}")
    nc.scalar.activation(T_g, P[:, 1536:2048], AF.Tanh)

    t_fc = spool.tile([128, 512], F32, name=f"t_fc_u{u}", tag=f"t_fc_u{u}")
    nc.vector.tensor_mul(t_fc, S_ifo[:, 512:1024], c_r)
    t_ig = spool.tile([128, 512], BF16, name=f"t_ig_u{u}", tag=f"t_ig_u{u}")
    nc.vector.tensor_mul(t_ig, S_ifo[:, 0:512], T_g)
    nc.vector.tensor_add(c_w, t_fc, t_ig)
    T_c = spool.tile([128, 512], BF16, name=f"T_c_u{u}", tag=f"T_c_u{u}")
    nc.scalar.activation(T_c, c_w, AF.Tanh)
    nc.vector.tensor_mul(h_w, S_ifo[:, 1024:1536], T_c)

    if s < n_stage - 1:
        tn = s + 1
        nc.gpsimd.tensor_copy(xh_w[0:64, :], h_w[0:64, :])
        nc.sync.dma_start(
            xh_w[64:72, :],
            cst.xT[
                (tn % 16) * 8 : (tn % 16) * 8 + 8,
                (tn // 16) * BC + u * BSUB : (tn // 16) * BC + (u + 1) * BSUB,
            ],
        )
    else:
        # final: logits = h1_{T-1} @ Wfc.T + bfc ; sigmoid
        P_fc = gpool.tile([1, BSUB], F32, name=f"P_fc_u{u}", tag=f"P_u{u}")
        nc.tensor.matmul(P_fc, cst.wfc, h_w, start=True, stop=True)
        S_out = spool.tile([1, BSUB], F32, name=f"S_out_u{u}", tag=f"S_out_u{u}")
        nc.scalar.activation(S_out, P_fc, AF.Sigmoid, bias=cst.bfc[0:1, 0:1])
        nc.sync.dma_start(cst.out_d[u * BSUB : (u + 1) * BSUB, :], S_out)


def _build_module(t_steps=T_FULL):
    assert t_steps % 16 == 0
    n_stage = t_steps + 1
    n_ctx0 = min(128, n_stage - 1)
    nc = bacc.Bacc("TRN2", target_bir_lowering=False, debug=False, enable_asserts=False)
    x_d = nc.dram_tensor("x", [BC, t_steps * D], F32, kind="ExternalInput").ap()
    w0big_d = nc.dram_tensor("w0big", [73, 512], BF16, kind="ExternalInput").ap()
    w0s0_d = nc.dram_tensor("w0s0", [73, 512], BF16, kind="ExternalInput").ap()
    w1_d = nc.dram_tensor("w1", [128, 256], BF16, kind="ExternalInput").ap()
    wfc_d = nc.dram_tensor("wfc", [128, 1], BF16, kind="ExternalInput").ap()
    bfc_d = nc.dram_tensor("bfc", [1, 1], F32, kind="ExternalInput").ap()
    id_d = nc.dram_tensor("ident", [128, 128], F32, kind="ExternalInput").ap()
    out_d = nc.dram_tensor("out", [BC, 1], F32, kind="ExternalOutput").ap()

    n_tc = t_steps // 16
    cst = _Consts()
    cst.w0big = nc.alloc_sbuf_tensor("w0big_sb", [73, 512], BF16).ap()
    cst.w0s0 = nc.alloc_sbuf_tensor("w0s0_sb", [73, 512], BF16).ap()
    cst.w1 = nc.alloc_sbuf_tensor("w1_sb", [128, 256], BF16).ap()
    cst.wfc = nc.alloc_sbuf_tensor("wfc_sb", [128, 1], BF16).ap()
    cst.bfc = nc.alloc_sbuf_tensor("bfc_sb", [1, 1], F32).ap()
    cst.xT = nc.alloc_sbuf_tensor("xT_sb", [128, (n_tc + 1) * BC], BF16).ap()
    cst.out_d = out_d
    # persistent ping-pong LSTM state
    cst.h = [
        [nc.alloc_sbuf_tensor(f"h_u{u}_p{p}", [128, BSUB], BF16).ap() for p in range(2)]
        for u in range(NSUB)
    ]
    cst.c = [
        [nc.alloc_sbuf_tensor(f"c_u{u}_p{p}", [128, BSUB], F32).ap() for p in range(2)]
        for u in range(NSUB)
    ]
    cst.xh = [
        [nc.alloc_sbuf_tensor(f"xh_u{u}_p{p}", [73, BSUB], BF16).ap() for p in range(2)]
        for u in range(NSUB)
    ]

    # ---- context 0: consts, transpose, stages 0..n_ctx0-1 ----
    with tile.TileContext(nc) as tc:
        with tc.sbuf_pool(name="state0", bufs=2) as spool, tc.sbuf_pool(
            name="cload", bufs=1
        ) as cpool:
            ident_sb = cpool.tile([128, 128], F32, name="ident_sb", tag="ident_sb")
            nc.sync.dma_start(ident_sb, id_d)
            nc.sync.dma_start(cst.w0big, w0big_d)
            nc.sync.dma_start(cst.w0s0, w0s0_d)
            nc.sync.dma_start(cst.w1, w1_d)
            nc.sync.dma_start(cst.wfc, wfc_d)
            nc.sync.dma_start(cst.bfc, bfc_d)

            _emit_transpose_phase(nc, tc, x_d, ident_sb, cst.xT, t_steps)

            for u in range(NSUB):
                nc.gpsimd.memset(cst.h[u][0], 0.0)
                nc.gpsimd.memset(cst.c[u][0], 0.0)
                nc.gpsimd.memset(cst.xh[u][0][0:64, :], 0.0)
                nc.vector.memset(cst.xh[u][0][72:73, :], 1.0)
                nc.vector.memset(cst.xh[u][1][72:73, :], 1.0)
                nc.sync.dma_start(
                    cst.xh[u][0][64:72, :], cst.xT[0:8, u * BSUB : (u + 1) * BSUB]
                )

            with tc.psum_pool(name="pg0", bufs=1) as gpool:
                for s in range(n_ctx0):
                    for u in range(NSUB):
                        _emit_stage(nc, spool, gpool, cst, s, u, n_stage)

    # ---- context 1: stages n_ctx0..n_stage-1 + fc ----
    with tile.TileContext(nc) as tc:
        with tc.sbuf_pool(name="state1", bufs=2) as spool:
            with tc.psum_pool(name="pg1", bufs=1) as gpool:
                for s in range(n_ctx0, n_stage):
                    for u in range(NSUB):
                        _emit_stage(nc, spool, gpool, cst, s, u, n_stage)

    nc.compile()
    return nc


def _prep_weights(Wih0, Whh0, bih0, bhh0, Wih1, Whh1, bih1, bhh1, Wfc, bfc):
    bf = ml_dtypes.bfloat16
    b0 = (bih0 + bhh0).astype(np.float32)
    b1 = (bih1 + bhh1).astype(np.float32)
    w0big = np.zeros((73, 512), np.float32)
    w1 = np.zeros((128, 256), np.float32)
    for j, g in enumerate(GATE_SLICES):
        r = slice(g * 64, (g + 1) * 64)
        w0big[0:64, j * 128 : j * 128 + 64] = Whh0[r].T
        w0big[64:72, j * 128 : j * 128 + 64] = Wih0[r].T
        w0big[72, j * 128 : j * 128 + 64] = b0[r]
        w0big[72, j * 128 + 64 : j * 128 + 128] = b1[r]
        w1[0:64, j * 64 : (j + 1) * 64] = Wih1[r].T
        w1[64:128, j * 64 : (j + 1) * 64] = Whh1[r].T
    w0s0 = w0big.copy()
    for j in range(4):
        w0s0[72, j * 128 + 64 : j * 128 + 128] = 0.0
    wfc = np.concatenate(
        [np.zeros((64, 1), np.float32), Wfc.reshape(1, 64).T], axis=0
    )
    ident = np.eye(128, dtype=np.float32)
    return (
        w0big.astype(bf),
        w0s0.astype(bf),
        w1.astype(bf),
        wfc.astype(bf),
        np.array([[bfc[0]]], dtype=np.float32),
        ident,
    )


_MODULE_CACHE = {}


def _get_module(t_steps=T_FULL):
    if t_steps not in _MODULE_CACHE:
        _MODULE_CACHE[t_steps] = _build_module(t_steps)
    return _MODULE_CACHE[t_steps]


def _run(inputs, trace=False, **spmd_kwargs):
    x = np.asarray(inputs["x"], np.float32)
    w0big, w0s0, w1, wfc, bfc, ident = _prep_weights(
        np.asarray(inputs["Wih0"], np.float32),
        np.asarray(inputs["Whh0"], np.float32),
        np.asarray(inputs["bih0"], np.float32),
        np.asarray(inputs["bhh0"], np.float32),
        np.asarray(inputs["Wih1"], np.float32),
        np.asarray(inputs["Whh1"], np.float32),
        np.asarray(inputs["bih1"], np.float32),
        np.asarray(inputs["bhh1"], np.float32),
        np.asarray(inputs["Wfc"], np.float32),
        np.asarray(inputs["bfc"], np.float32),
    )
    nc = _get_module(T_FULL)
    in_maps = []
    for c in range(N_CORES):
        shard = np.ascontiguousarray(
            x[c * BC : (c + 1) * BC].reshape(BC, T_FULL * D)
        )
        in_maps.append(
            {
                "x": shard,
                "w0big": w0big,
                "w0s0": w0s0,
                "w1": w1,
                "wfc": wfc,
                "bfc": bfc,
                "ident": ident,
            }
        )
    res = run_bass_kernel_spmd(
        nc, in_maps, core_ids=list(range(N_CORES)), trace=trace, **spmd_kwargs
    )
    out = np.concatenate(
        [res.results[c]["out"] for c in range(N_CORES)], axis=0
    ).astype(np.float32)
    return out, res


def kernel(**inputs):
    out, _ = _run(inputs, trace=False)
    return out


# revision 4
# speedup vs baseline: 1.3381x; 1.3381x over previous
"""Trainium2 Bass kernel: 2-layer LSTM (H=64, D=8, T=256) + FC head, batch 8192.

v4: per-gate M=64 matmul pairs (full-clock under the PE power governor; M=128
mms get p-state-throttled to 1.2GHz), per-gate ACT instructions with
per-partition bias [b0_j; b1_j], plus the v3 wins:
  - x-slice DMA prefetched one stage ahead into persistent ping-pong xh tiles
    (off the critical path); h0->xh copy on the otherwise-idle GPSIMD.
  - bf16 cell state: all four DVE tensor ops hit the 2x 16-bit mode (~423ns).
  - x transposes (PE, via identity) interleaved INTO the stage stream two
    14-timestep blocks ahead, borrowing stage PSUM tags; copies on DVE.
  - ACT emission rotated so T_c of unit 0 sits after unit 1's first gate
    instruction: no in-order-queue slip on the c' -> tanh(c) edge.
  - Stage 0 uses a bias column with the layer1 half zeroed so the layer1 half
    computes exactly 0 (= h1_{-1}); stage T computes a garbage-but-finite
    layer0 step while layer1 finishes h1_{T-1} for the FC head.
  - 257 stages split across two TileContexts (event-sem saturation).
"""

import numpy as np
import ml_dtypes

import concourse.bass as bass
import concourse.bacc as bacc
import concourse.mybir as mybir
import concourse.tile as tile
from concourse.bass_utils import run_bass_kernel_spmd

F32 = mybir.dt.float32
BF16 = mybir.dt.bfloat16
AF = mybir.ActivationFunctionType

H = 64
D = 8
T_FULL = 256
B_TOTAL = 8192
N_CORES = 8
BC = B_TOTAL // N_CORES  # 1024 per core
NSUB = 2
BSUB = BC // NSUB  # 512

SLOT = 9  # x dims (8) + ones col in x2 (ones unused in v4)
TPB = 14  # timesteps per xTones block
NBLK = 19
XCOLS = NBLK * TPB * SLOT  # 2394

# gate order in PSUM banks / ACT emission: i, g, f, o (ref rows: i=0,f=1,g=2,o=3)
GORDER = [0, 2, 1, 3]
GFUNC = [AF.Sigmoid, AF.Tanh, AF.Sigmoid, AF.Sigmoid]


class _Consts:
    pass


def _emit_block_transpose(nc, gpool, cst, blk, tag):
    pt = gpool.tile([126, 1024], F32, name=f"pt_b{blk}", tag=tag)
    for ch in range(8):
        nc.tensor.transpose(
            pt[:, ch * 128 : (ch + 1) * 128],
            cst.xs[ch][:, blk * 126 : (blk + 1) * 126],
            cst.ident,
        )
    nc.vector.tensor_copy(cst.xTones[:, blk * BC : (blk + 1) * BC], pt)


def _emit_stage(nc, spool, gpool, cst, s, n_stage):
    rp = s % 2
    wp = (s + 1) % 2
    blk, slot = s // TPB, s % TPB
    bcol = 4 if s == 0 else 0  # stage-0 bias columns have the l1 half zeroed

    # prefetch next stage's x-slice into xh rows 64:72 (off the critical path)
    if s + 1 < n_stage:
        blk1, slot1 = (s + 1) // TPB, (s + 1) % TPB
        for u in range(NSUB):
            nc.sync.dma_start(
                cst.xh[u][wp][64:72, :],
                cst.xTones[
                    slot1 * SLOT : slot1 * SLOT + 8,
                    blk1 * BC + u * BSUB : blk1 * BC + (u + 1) * BSUB,
                ],
            )

    # interleaved transpose of block blk+2
    if slot == 7 and blk + 2 < NBLK:
        _emit_block_transpose(nc, gpool, cst, blk + 2, f"P_u{(blk + 2) % NSUB}")

    # matmuls in gate order f,i,g,o (S_f is consumed first by the DVE chain)
    MMORD = [2, 0, 1, 3]
    P = []
    for u in range(NSUB):
        Pu = gpool.tile([128, 2048], F32, name=f"P_u{u}", tag=f"P_u{u}")
        P.append(Pu)
        for j in MMORD:
            # layer1: rhs = stacked h directly (no staging), rows 64:128
            nc.tensor.matmul(
                Pu[64:128, j * 512 : (j + 1) * 512],
                cst.w1[:, j * 64 : (j + 1) * 64],
                cst.h[u][rp],
                start=True,
                stop=True,
            )
            # layer0: rhs = xh [h0 copy; x slice], rows 0:64
            nc.tensor.matmul(
                Pu[0:64, j * 512 : (j + 1) * 512],
                cst.w0[:, j * 64 : (j + 1) * 64],
                cst.xh[u][rp],
                start=True,
                stop=True,
            )

    # explicit ACT/DVE braid for NSUB=2: gate acts in order f,i,g,o; each
    # unit's T_c is rotated to land after the other unit's first gate act so
    # the in-order ACT queue never stalls on the DVE c' edge.
    def gate_act(u, j):
        t = spool.tile([128, 512], BF16, name=f"S{j}_u{u}", tag=f"S{j}_u{u}")
        nc.scalar.activation(
            t, P[u][:, j * 512 : (j + 1) * 512], GFUNC[j],
            bias=cst.biases[:, bcol + j : bcol + j + 1],
        )
        return t

    def dve_cchain(u, S):
        t_fc = spool.tile([128, 512], BF16, name=f"t_fc_u{u}", tag=f"t_fc_u{u}")
        nc.vector.tensor_mul(t_fc, S[2], cst.c[u][rp])
        t_ig = spool.tile([128, 512], BF16, name=f"t_ig_u{u}", tag=f"t_ig_u{u}")
        nc.vector.tensor_mul(t_ig, S[0], S[1])
        nc.vector.tensor_add(cst.c[u][wp], t_fc, t_ig)

    def finish_unit(u, S):
        T_c = spool.tile([128, 512], BF16, name=f"T_c_u{u}", tag=f"T_c_u{u}")
        nc.scalar.activation(T_c, cst.c[u][wp], AF.Tanh)
        nc.vector.tensor_mul(cst.h[u][wp], S[3], T_c)
        if s + 1 < n_stage:
            nc.vector.tensor_copy(cst.xh[u][wp][0:64, :], cst.h[u][wp][0:64, :])

    S0 = [None] * 4
    S1 = [None] * 4
    for j in MMORD:
        S0[j] = gate_act(0, j)
    dve_cchain(0, S0)
    S1[2] = gate_act(1, 2)  # S_f_u1
    finish_unit(0, S0)      # T_c_u0, h_u0, xh copy_u0
    for j in (0, 1, 3):
        S1[j] = gate_act(1, j)
    dve_cchain(1, S1)
    finish_unit(1, S1)

    if s == n_stage - 1:
        for u in range(NSUB):
            P_fc = gpool.tile([1, BSUB], F32, name=f"P_fc_u{u}", tag=f"P_u{u}")
            nc.tensor.matmul(P_fc, cst.wfc, cst.h[u][wp], start=True, stop=True)
            S_out = spool.tile([1, BSUB], F32, name=f"S_out_u{u}", tag=f"S_out_u{u}")
            nc.scalar.activation(S_out, P_fc, AF.Sigmoid, bias=cst.bfc[0:1, 0:1])
            nc.sync.dma_start(cst.out_d[u * BSUB : (u + 1) * BSUB, :], S_out)


def _build_module(t_steps=T_FULL):
    n_stage = t_steps + 1
    n_ctx0 = 128
    nc = bacc.Bacc("TRN2", target_bir_lowering=False, debug=False, enable_asserts=False)
    x2_d = nc.dram_tensor("x2", [BC, XCOLS], F32, kind="ExternalInput").ap()
    w0_d = nc.dram_tensor("w0", [72, 256], BF16, kind="ExternalInput").ap()
    w1_d = nc.dram_tensor("w1", [128, 256], BF16, kind="ExternalInput").ap()
    wfc_d = nc.dram_tensor("wfc", [128, 1], BF16, kind="ExternalInput").ap()
    bias_d = nc.dram_tensor("biases", [128, 8], F32, kind="ExternalInput").ap()
    bfc_d = nc.dram_tensor("bfc", [1, 1], F32, kind="ExternalInput").ap()
    id_d = nc.dram_tensor("ident", [128, 128], F32, kind="ExternalInput").ap()
    out_d = nc.dram_tensor("out", [BC, 1], F32, kind="ExternalOutput").ap()

    cst = _Consts()
    cst.w0 = nc.alloc_sbuf_tensor("w0_sb", [72, 256], BF16).ap()
    cst.w1 = nc.alloc_sbuf_tensor("w1_sb", [128, 256], BF16).ap()
    cst.wfc = nc.alloc_sbuf_tensor("wfc_sb", [128, 1], BF16).ap()
    cst.biases = nc.alloc_sbuf_tensor("bias_sb", [128, 8], F32).ap()
    cst.bfc = nc.alloc_sbuf_tensor("bfc_sb", [1, 1], F32).ap()
    cst.ident = nc.alloc_sbuf_tensor("ident_sb", [128, 128], F32).ap()
    cst.xTones = nc.alloc_sbuf_tensor("xTones_sb", [126, NBLK * BC], BF16).ap()
    cst.xs = [
        nc.alloc_sbuf_tensor(f"xs{ch}", [128, XCOLS], F32).ap() for ch in range(8)
    ]
    cst.out_d = out_d
    cst.h = [
        [nc.alloc_sbuf_tensor(f"h_u{u}_p{p}", [128, BSUB], BF16).ap() for p in range(2)]
        for u in range(NSUB)
    ]
    cst.c = [
        [nc.alloc_sbuf_tensor(f"c_u{u}_p{p}", [128, BSUB], BF16).ap() for p in range(2)]
        for u in range(NSUB)
    ]
    cst.xh = [
        [nc.alloc_sbuf_tensor(f"xh_u{u}_p{p}", [72, BSUB], BF16).ap() for p in range(2)]
        for u in range(NSUB)
    ]

    EARLY = 3 * 126

    with tile.TileContext(nc) as tc:
        with tc.sbuf_pool(name="state0", bufs=2) as spool:
            nc.sync.dma_start(cst.ident, id_d)
            nc.sync.dma_start(cst.w0, w0_d)
            nc.sync.dma_start(cst.w1, w1_d)
            nc.sync.dma_start(cst.wfc, wfc_d)
            nc.sync.dma_start(cst.biases, bias_d)
            nc.sync.dma_start(cst.bfc, bfc_d)
            for ch in range(8):
                nc.sync.dma_start(
                    cst.xs[ch][:, 0:EARLY], x2_d[ch * 128 : (ch + 1) * 128, 0:EARLY]
                )
            for ch in range(8):
                nc.sync.dma_start(
                    cst.xs[ch][:, EARLY:XCOLS],
                    x2_d[ch * 128 : (ch + 1) * 128, EARLY:XCOLS],
                )
            for u in range(NSUB):
                nc.gpsimd.memset(cst.h[u][0], 0.0)
                nc.gpsimd.memset(cst.c[u][0], 0.0)
                nc.gpsimd.memset(cst.xh[u][0][0:64, :], 0.0)

            with tc.psum_pool(name="pg0", bufs=1) as gpool:
                _emit_block_transpose(nc, gpool, cst, 0, "P_u0")
                _emit_block_transpose(nc, gpool, cst, 1, "P_u1")
                for u in range(NSUB):
                    nc.sync.dma_start(
                        cst.xh[u][0][64:72, :],
                        cst.xTones[0:8, u * BSUB : (u + 1) * BSUB],
                    )
                for s in range(n_ctx0):
                    _emit_stage(nc, spool, gpool, cst, s, n_stage)

    with tile.TileContext(nc) as tc:
        with tc.sbuf_pool(name="state1", bufs=2) as spool:
            with tc.psum_pool(name="pg1", bufs=1) as gpool:
                for s in range(n_ctx0, n_stage):
                    _emit_stage(nc, spool, gpool, cst, s, n_stage)

    nc.compile()
    return nc


def _prep_weights(Wih0, Whh0, bih0, bhh0, Wih1, Whh1, bih1, bhh1, Wfc, bfc):
    bf = ml_dtypes.bfloat16
    b0 = (bih0 + bhh0).astype(np.float32)
    b1 = (bih1 + bhh1).astype(np.float32)
    w0 = np.zeros((72, 256), np.float32)
    w1 = np.zeros((128, 256), np.float32)
    biases = np.zeros((128, 8), np.float32)
    for j, g in enumerate(GORDER):
        r = slice(g * 64, (g + 1) * 64)
        w0[0:64, j * 64 : (j + 1) * 64] = Whh0[r].T
        w0[64:72, j * 64 : (j + 1) * 64] = Wih0[r].T
        w1[0:64, j * 64 : (j + 1) * 64] = Wih1[r].T
        w1[64:128, j * 64 : (j + 1) * 64] = Whh1[r].T
        biases[0:64, j] = b0[r]
        biases[64:128, j] = b1[r]
        biases[0:64, 4 + j] = b0[r]  # stage-0 columns: l1 half stays 0
    wfc = np.concatenate(
        [np.zeros((64, 1), np.float32), Wfc.reshape(1, 64).T], axis=0
    )
    ident = np.eye(128, dtype=np.float32)
    return (
        w0.astype(bf),
        w1.astype(bf),
        wfc.astype(bf),
        biases,
        np.array([[bfc[0]]], dtype=np.float32),
        ident,
    )


def _prep_x2(x):
    B = x.shape[0]
    x2 = np.zeros((B, NBLK * TPB, SLOT), np.float32)
    x2[:, :T_FULL, 0:D] = x
    x2[:, :, D] = 1.0
    return np.ascontiguousarray(x2.reshape(B, XCOLS))


_MODULE_CACHE = {}


def _get_module(t_steps=T_FULL):
    if t_steps not in _MODULE_CACHE:
        _MODULE_CACHE[t_steps] = _build_module(t_steps)
    return _MODULE_CACHE[t_steps]


def _run(inputs, trace=False, **spmd_kwargs):
    x = np.asarray(inputs["x"], np.float32)
    w0, w1, wfc, biases, bfc, ident = _prep_weights(
        np.asarray(inputs["Wih0"], np.float32),
        np.asarray(inputs["Whh0"], np.float32),
        np.asarray(inputs["bih0"], np.float32),
        np.asarray(inputs["bhh0"], np.float32),
        np.asarray(inputs["Wih1"], np.float32),
        np.asarray(inputs["Whh1"], np.float32),
        np.asarray(inputs["bih1"], np.float32),
        np.asarray(inputs["bhh1"], np.float32),
        np.asarray(inputs["Wfc"], np.float32),
        np.asarray(inputs["bfc"], np.float32),
    )
    x2 = _prep_x2(x)
    nc = _get_module(T_FULL)
    in_maps = []
    for c in range(N_CORES):
        in_maps.append(
            {
                "x2": x2[c * BC : (c + 1) * BC],
                "w0": w0,
                "w1": w1,
                "wfc": wfc,
                "biases": biases,
                "bfc": bfc,
                "ident": ident,
            }
        )
    res = run_bass_kernel_spmd(
        nc, in_maps, core_ids=list(range(N_CORES)), trace=trace, **spmd_kwargs
    )
    out = np.concatenate(
        [res.results[c]["out"] for c in range(N_CORES)], axis=0
    ).astype(np.float32)
    return out, res


def kernel(**inputs):
    out, _ = _run(inputs, trace=False)
    return out


# revision 5
# speedup vs baseline: 1.3386x; 1.0004x over previous
"""Trainium2 Bass kernel: 2-layer LSTM (H=64, D=8, T=256) + FC head, batch 8192.

v4: per-gate M=64 matmul pairs (full-clock under the PE power governor; M=128
mms get p-state-throttled to 1.2GHz), per-gate ACT instructions with
per-partition bias [b0_j; b1_j], plus the v3 wins:
  - x-slice DMA prefetched one stage ahead into persistent ping-pong xh tiles
    (off the critical path); h0->xh copy on the otherwise-idle GPSIMD.
  - bf16 cell state: all four DVE tensor ops hit the 2x 16-bit mode (~423ns).
  - x transposes (PE, via identity) interleaved INTO the stage stream two
    14-timestep blocks ahead, borrowing stage PSUM tags; copies on DVE.
  - ACT emission rotated so T_c of unit 0 sits after unit 1's first gate
    instruction: no in-order-queue slip on the c' -> tanh(c) edge.
  - Stage 0 uses a bias column with the layer1 half zeroed so the layer1 half
    computes exactly 0 (= h1_{-1}); stage T computes a garbage-but-finite
    layer0 step while layer1 finishes h1_{T-1} for the FC head.
  - 257 stages split across two TileContexts (event-sem saturation).
"""

import numpy as np
import ml_dtypes

import concourse.bass as bass
import concourse.bacc as bacc
import concourse.mybir as mybir
import concourse.tile as tile
from concourse.bass_utils import run_bass_kernel_spmd

F32 = mybir.dt.float32
BF16 = mybir.dt.bfloat16
AF = mybir.ActivationFunctionType

H = 64
D = 8
T_FULL = 256
B_TOTAL = 8192
N_CORES = 8
BC = B_TOTAL // N_CORES  # 1024 per core
NSUB = 2
BSUB = BC // NSUB  # 512

SLOT = 9  # x dims (8) + ones col in x2 (ones unused in v4)
TPB = 14  # timesteps per xTones block
NBLK = 19
XCOLS = NBLK * TPB * SLOT  # 2394

# gate order in PSUM banks / ACT emission: i, g, f, o (ref rows: i=0,f=1,g=2,o=3)
GORDER = [0, 2, 1, 3]
GFUNC = [AF.Sigmoid, AF.Tanh, AF.Sigmoid, AF.Sigmoid]


class _Consts:
    pass


def _emit_block_transpose(nc, gpool, cst, blk, tag):
    pt = gpool.tile([126, 1024], F32, name=f"pt_b{blk}", tag=tag)
    for ch in range(8):
        nc.tensor.transpose(
            pt[:, ch * 128 : (ch + 1) * 128],
            cst.xs[ch][:, blk * 126 : (blk + 1) * 126],
            cst.ident,
        )
    nc.vector.tensor_copy(cst.xTones[:, blk * BC : (blk + 1) * BC], pt)


def _emit_stage(nc, spool, gpool, cst, s, n_stage):
    rp = s % 2
    wp = (s + 1) % 2
    blk, slot = s // TPB, s % TPB
    bcol = 4 if s == 0 else 0  # stage-0 bias columns have the l1 half zeroed

    # prefetch next stage's x-slice into xh rows 64:72 (off the critical path)
    if s + 1 < n_stage:
        blk1, slot1 = (s + 1) // TPB, (s + 1) % TPB
        for u in range(NSUB):
            nc.sync.dma_start(
                cst.xh[u][wp][64:72, :],
                cst.xTones[
                    slot1 * SLOT : slot1 * SLOT + 8,
                    blk1 * BC + u * BSUB : blk1 * BC + (u + 1) * BSUB,
                ],
            )

    # interleaved transpose of block blk+2
    if slot == 7 and blk + 2 < NBLK:
        _emit_block_transpose(nc, gpool, cst, blk + 2, f"P_u{(blk + 2) % NSUB}")

    # matmuls in gate order f,i,g,o (S_f is consumed first by the DVE chain)
    MMORD = [2, 0, 1, 3]
    P = []
    for u in range(NSUB):
        Pu = gpool.tile([128, 2048], F32, name=f"P_u{u}", tag=f"P_u{u}")
        P.append(Pu)
        for j in MMORD:
            # layer1: rhs = stacked h directly (no staging), rows 64:128
            nc.tensor.matmul(
                Pu[64:128, j * 512 : (j + 1) * 512],
                cst.w1[:, j * 64 : (j + 1) * 64],
                cst.h[u][rp],
                start=True,
                stop=True,
            )
            # layer0: rhs = xh [h0 copy; x slice], rows 0:64
            nc.tensor.matmul(
                Pu[0:64, j * 512 : (j + 1) * 512],
                cst.w0[:, j * 64 : (j + 1) * 64],
                cst.xh[u][rp],
                start=True,
                stop=True,
            )

    # explicit ACT/DVE braid for NSUB=2: gate acts in order f,i,g,o; each
    # unit's T_c is rotated to land after the other unit's first gate act so
    # the in-order ACT queue never stalls on the DVE c' edge.
    def gate_act(u, j):
        t = spool.tile([128, 512], BF16, name=f"S{j}_u{u}", tag=f"S{j}_u{u}")
        nc.scalar.activation(
            t, P[u][:, j * 512 : (j + 1) * 512], GFUNC[j],
            bias=cst.biases[:, bcol + j : bcol + j + 1],
        )
        return t

    def dve_cchain(u, S):
        t_fc = spool.tile([128, 512], BF16, name=f"t_fc_u{u}", tag=f"t_fc_u{u}")
        nc.vector.tensor_mul(t_fc, S[2], cst.c[u][rp])
        t_ig = spool.tile([128, 512], BF16, name=f"t_ig_u{u}", tag=f"t_ig_u{u}")
        nc.vector.tensor_mul(t_ig, S[0], S[1])
        nc.vector.tensor_add(cst.c[u][wp], t_fc, t_ig)

    def finish_unit(u, S):
        T_c = spool.tile([128, 512], BF16, name=f"T_c_u{u}", tag=f"T_c_u{u}")
        nc.scalar.activation(T_c, cst.c[u][wp], AF.Tanh)
        nc.vector.tensor_mul(cst.h[u][wp], S[3], T_c)
        if s + 1 < n_stage:
            nc.vector.tensor_copy(cst.xh[u][wp][0:64, :], cst.h[u][wp][0:64, :])

    S0 = [None] * 4
    S1 = [None] * 4
    for j in MMORD:
        S0[j] = gate_act(0, j)
    dve_cchain(0, S0)
    finish_unit(0, S0)  # T_c_u0 immediately after u0's c': starts the
    for j in MMORD:     # h0->copy->matmul tail a full ACT slot earlier
        S1[j] = gate_act(1, j)
    dve_cchain(1, S1)
    finish_unit(1, S1)

    if s == n_stage - 1:
        for u in range(NSUB):
            P_fc = gpool.tile([1, BSUB], F32, name=f"P_fc_u{u}", tag=f"P_u{u}")
            nc.tensor.matmul(P_fc, cst.wfc, cst.h[u][wp], start=True, stop=True)
            S_out = spool.tile([1, BSUB], F32, name=f"S_out_u{u}", tag=f"S_out_u{u}")
            nc.scalar.activation(S_out, P_fc, AF.Sigmoid, bias=cst.bfc[0:1, 0:1])
            nc.sync.dma_start(cst.out_d[u * BSUB : (u + 1) * BSUB, :], S_out)


def _build_module(t_steps=T_FULL):
    n_stage = t_steps + 1
    n_ctx0 = 128
    nc = bacc.Bacc("TRN2", target_bir_lowering=False, debug=False, enable_asserts=False)
    x2_d = nc.dram_tensor("x2", [BC, XCOLS], F32, kind="ExternalInput").ap()
    w0_d = nc.dram_tensor("w0", [72, 256], BF16, kind="ExternalInput").ap()
    w1_d = nc.dram_tensor("w1", [128, 256], BF16, kind="ExternalInput").ap()
    wfc_d = nc.dram_tensor("wfc", [128, 1], BF16, kind="ExternalInput").ap()
    bias_d = nc.dram_tensor("biases", [128, 8], F32, kind="ExternalInput").ap()
    bfc_d = nc.dram_tensor("bfc", [1, 1], F32, kind="ExternalInput").ap()
    id_d = nc.dram_tensor("ident", [128, 128], F32, kind="ExternalInput").ap()
    out_d = nc.dram_tensor("out", [BC, 1], F32, kind="ExternalOutput").ap()

    cst = _Consts()
    cst.w0 = nc.alloc_sbuf_tensor("w0_sb", [72, 256], BF16).ap()
    cst.w1 = nc.alloc_sbuf_tensor("w1_sb", [128, 256], BF16).ap()
    cst.wfc = nc.alloc_sbuf_tensor("wfc_sb", [128, 1], BF16).ap()
    cst.biases = nc.alloc_sbuf_tensor("bias_sb", [128, 8], F32).ap()
    cst.bfc = nc.alloc_sbuf_tensor("bfc_sb", [1, 1], F32).ap()
    cst.ident = nc.alloc_sbuf_tensor("ident_sb", [128, 128], F32).ap()
    cst.xTones = nc.alloc_sbuf_tensor("xTones_sb", [126, NBLK * BC], BF16).ap()
    cst.xs = [
        nc.alloc_sbuf_tensor(f"xs{ch}", [128, XCOLS], F32).ap() for ch in range(8)
    ]
    cst.out_d = out_d
    cst.h = [
        [nc.alloc_sbuf_tensor(f"h_u{u}_p{p}", [128, BSUB], BF16).ap() for p in range(2)]
        for u in range(NSUB)
    ]
    cst.c = [
        [nc.alloc_sbuf_tensor(f"c_u{u}_p{p}", [128, BSUB], BF16).ap() for p in range(2)]
        for u in range(NSUB)
    ]
    cst.xh = [
        [nc.alloc_sbuf_tensor(f"xh_u{u}_p{p}", [72, BSUB], BF16).ap() for p in range(2)]
        for u in range(NSUB)
    ]

    EARLY = 3 * 126

    with tile.TileContext(nc) as tc:
        with tc.sbuf_pool(name="state0", bufs=2) as spool:
            nc.sync.dma_start(cst.ident, id_d)
            nc.sync.dma_start(cst.w0, w0_d)
            nc.sync.dma_start(cst.w1, w1_d)
            nc.sync.dma_start(cst.wfc, wfc_d)
            nc.sync.dma_start(cst.biases, bias_d)
            nc.sync.dma_start(cst.bfc, bfc_d)
            for ch in range(8):
                nc.sync.dma_start(
                    cst.xs[ch][:, 0:EARLY], x2_d[ch * 128 : (ch + 1) * 128, 0:EARLY]
                )
            for ch in range(8):
                nc.sync.dma_start(
                    cst.xs[ch][:, EARLY:XCOLS],
                    x2_d[ch * 128 : (ch + 1) * 128, EARLY:XCOLS],
                )
            for u in range(NSUB):
                nc.gpsimd.memset(cst.h[u][0], 0.0)
                nc.gpsimd.memset(cst.c[u][0], 0.0)
                nc.gpsimd.memset(cst.xh[u][0][0:64, :], 0.0)

            with tc.psum_pool(name="pg0", bufs=1) as gpool:
                _emit_block_transpose(nc, gpool, cst, 0, "P_u0")
                _emit_block_transpose(nc, gpool, cst, 1, "P_u1")
                for u in range(NSUB):
                    nc.sync.dma_start(
                        cst.xh[u][0][64:72, :],
                        cst.xTones[0:8, u * BSUB : (u + 1) * BSUB],
                    )
                for s in range(n_ctx0):
                    _emit_stage(nc, spool, gpool, cst, s, n_stage)

    with tile.TileContext(nc) as tc:
        with tc.sbuf_pool(name="state1", bufs=2) as spool:
            with tc.psum_pool(name="pg1", bufs=1) as gpool:
                for s in range(n_ctx0, n_stage):
                    _emit_stage(nc, spool, gpool, cst, s, n_stage)

    nc.compile()
    return nc


def _prep_weights(Wih0, Whh0, bih0, bhh0, Wih1, Whh1, bih1, bhh1, Wfc, bfc):
    bf = ml_dtypes.bfloat16
    b0 = (bih0 + bhh0).astype(np.float32)
    b1 = (bih1 + bhh1).astype(np.float32)
    w0 = np.zeros((72, 256), np.float32)
    w1 = np.zeros((128, 256), np.float32)
    biases = np.zeros((128, 8), np.float32)
    for j, g in enumerate(GORDER):
        r = slice(g * 64, (g + 1) * 64)
        w0[0:64, j * 64 : (j + 1) * 64] = Whh0[r].T
        w0[64:72, j * 64 : (j + 1) * 64] = Wih0[r].T
        w1[0:64, j * 64 : (j + 1) * 64] = Wih1[r].T
        w1[64:128, j * 64 : (j + 1) * 64] = Whh1[r].T
        biases[0:64, j] = b0[r]
        biases[64:128, j] = b1[r]
        biases[0:64, 4 + j] = b0[r]  # stage-0 columns: l1 half stays 0
    wfc = np.concatenate(
        [np.zeros((64, 1), np.float32), Wfc.reshape(1, 64).T], axis=0
    )
    ident = np.eye(128, dtype=np.float32)
    return (
        w0.astype(bf),
        w1.astype(bf),
        wfc.astype(bf),
        biases,
        np.array([[bfc[0]]], dtype=np.float32),
        ident,
    )


def _prep_x2(x):
    B = x.shape[0]
    x2 = np.zeros((B, NBLK * TPB, SLOT), np.float32)
    x2[:, :T_FULL, 0:D] = x
    x2[:, :, D] = 1.0
    return np.ascontiguousarray(x2.reshape(B, XCOLS))


_MODULE_CACHE = {}


def _get_module(t_steps=T_FULL):
    if t_steps not in _MODULE_CACHE:
        _MODULE_CACHE[t_steps] = _build_module(t_steps)
    return _MODULE_CACHE[t_steps]


def _run(inputs, trace=False, **spmd_kwargs):
    x = np.asarray(inputs["x"], np.float32)
    w0, w1, wfc, biases, bfc, ident = _prep_weights(
        np.asarray(inputs["Wih0"], np.float32),
        np.asarray(inputs["Whh0"], np.float32),
        np.asarray(inputs["bih0"], np.float32),
        np.asarray(inputs["bhh0"], np.float32),
        np.asarray(inputs["Wih1"], np.float32),
        np.asarray(inputs["Whh1"], np.float32),
        np.asarray(inputs["bih1"], np.float32),
        np.asarray(inputs["bhh1"], np.float32),
        np.asarray(inputs["Wfc"], np.float32),
        np.asarray(inputs["bfc"], np.float32),
    )
    x2 = _prep_x2(x)
    nc = _get_module(T_FULL)
    in_maps = []
    for c in range(N_CORES):
        in_maps.append(
            {
                "x2": x2[c * BC : (c + 1) * BC],
                "w0": w0,
                "w1": w1,
                "wfc": wfc,
                "biases": biases,
                "bfc": bfc,
                "ident": ident,
            }
        )
    res = run_bass_kernel_spmd(
        nc, in_maps, core_ids=list(range(N_CORES)), trace=trace, **spmd_kwargs
    )
    out = np.concatenate(
        [res.results[c]["out"] for c in range(N_CORES)], axis=0
    ).astype(np.float32)
    return out, res


def kernel(**inputs):
    out, _ = _run(inputs, trace=False)
    return out


# revision 7
# speedup vs baseline: 1.3413x; 1.0020x over previous
"""Trainium2 Bass kernel: 2-layer LSTM (H=64, D=8, T=256) + FC head, batch 8192.

v4: per-gate M=64 matmul pairs (full-clock under the PE power governor; M=128
mms get p-state-throttled to 1.2GHz), per-gate ACT instructions with
per-partition bias [b0_j; b1_j], plus the v3 wins:
  - x-slice DMA prefetched one stage ahead into persistent ping-pong xh tiles
    (off the critical path); h0->xh copy on the otherwise-idle GPSIMD.
  - bf16 cell state: all four DVE tensor ops hit the 2x 16-bit mode (~423ns).
  - x transposes (PE, via identity) interleaved INTO the stage stream two
    14-timestep blocks ahead, borrowing stage PSUM tags; copies on DVE.
  - ACT emission rotated so T_c of unit 0 sits after unit 1's first gate
    instruction: no in-order-queue slip on the c' -> tanh(c) edge.
  - Stage 0 uses a bias column with the layer1 half zeroed so the layer1 half
    computes exactly 0 (= h1_{-1}); stage T computes a garbage-but-finite
    layer0 step while layer1 finishes h1_{T-1} for the FC head.
  - 257 stages split across two TileContexts (event-sem saturation).
"""

import numpy as np
import ml_dtypes

import concourse.bass as bass
import concourse.bacc as bacc
import concourse.mybir as mybir
import concourse.tile as tile
from concourse.bass_utils import run_bass_kernel_spmd

F32 = mybir.dt.float32
F8 = mybir.dt.float8e4
BF16 = mybir.dt.bfloat16
AF = mybir.ActivationFunctionType

H = 64
D = 8
T_FULL = 256
B_TOTAL = 8192
N_CORES = 8
BC = B_TOTAL // N_CORES  # 1024 per core
NSUB = 2
BSUB = BC // NSUB  # 512

SLOT = 9  # x dims (8) + ones col in x2 (ones unused in v4)
TPB = 14  # timesteps per xTones block
NBLK = 19
XCOLS = NBLK * TPB * SLOT  # 2394

# gate order in PSUM banks / ACT emission: i, g, f, o (ref rows: i=0,f=1,g=2,o=3)
GORDER = [0, 2, 1, 3]
GFUNC = [AF.Sigmoid, AF.Tanh, AF.Sigmoid, AF.Sigmoid]


class _Consts:
    pass


def _emit_block_transpose(nc, gpool, cst, blk, tag):
    pt = gpool.tile([126, 1024], F32, name=f"pt_b{blk}", tag=tag)
    for ch in range(8):
        nc.tensor.transpose(
            pt[:, ch * 128 : (ch + 1) * 128],
            cst.xs[ch][:, blk * 126 : (blk + 1) * 126],
            cst.ident,
        )
    nc.vector.tensor_copy(cst.xTones[:, blk * BC : (blk + 1) * BC], pt)


def _emit_stage(nc, spool, gpool, cst, s, n_stage):
    rp = s % 2
    wp = (s + 1) % 2
    blk, slot = s // TPB, s % TPB
    bcol = 4 if s == 0 else 0  # stage-0 bias columns have the l1 half zeroed

    # prefetch next stage's x-slice into xh rows 64:72 (off the critical path)
    if s + 1 < n_stage:
        blk1, slot1 = (s + 1) // TPB, (s + 1) % TPB
        for u in range(NSUB):
            nc.sync.dma_start(
                cst.xh[u][wp][64:72, :],
                cst.xTones[
                    slot1 * SLOT : slot1 * SLOT + 8,
                    blk1 * BC + u * BSUB : blk1 * BC + (u + 1) * BSUB,
                ],
            )

    # interleaved transpose of block blk+2
    if slot == 7 and blk + 2 < NBLK:
        _emit_block_transpose(nc, gpool, cst, blk + 2, f"P_u{(blk + 2) % NSUB}")

    # matmuls in gate order f,i,g,o (S_f is consumed first by the DVE chain)
    MMORD = [2, 0, 1, 3]
    P = []
    for u in range(NSUB):
        Pu = gpool.tile([128, 2048], F32, name=f"P_u{u}", tag=f"P_u{u}")
        P.append(Pu)
        for j in MMORD:
            # layer1: rhs = stacked h directly (no staging), rows 64:128
            nc.tensor.matmul(
                Pu[64:128, j * 512 : (j + 1) * 512],
                cst.w1[:, j * 64 : (j + 1) * 64],
                cst.h[u][rp],
                start=True,
                stop=True,
            )
            # layer0: rhs = xh [h0 copy; x slice], rows 0:64
            nc.tensor.matmul(
                Pu[0:64, j * 512 : (j + 1) * 512],
                cst.w0[:, j * 64 : (j + 1) * 64],
                cst.xh[u][rp],
                start=True,
                stop=True,
            )

    # explicit ACT/DVE braid for NSUB=2: gate acts in order f,i,g,o; each
    # unit's T_c is rotated to land after the other unit's first gate act so
    # the in-order ACT queue never stalls on the DVE c' edge.
    def gate_act(u, j):
        t = spool.tile([128, 512], BF16, name=f"S{j}_u{u}", tag=f"S{j}_u{u}")
        nc.scalar.activation(
            t, P[u][:, j * 512 : (j + 1) * 512], GFUNC[j],
            bias=cst.biases[:, bcol + j : bcol + j + 1],
        )
        return t

    def dve_cchain(u, S):
        t_fc = spool.tile([128, 512], BF16, name=f"t_fc_u{u}", tag=f"t_fc_u{u}")
        nc.vector.tensor_mul(t_fc, S[2], cst.c[u][rp])
        t_ig = spool.tile([128, 512], BF16, name=f"t_ig_u{u}", tag=f"t_ig_u{u}")
        nc.vector.tensor_mul(t_ig, S[0], S[1])
        nc.vector.tensor_add(cst.c[u][wp], t_fc, t_ig)

    def finish_unit(u, S):
        T_c = spool.tile([128, 512], BF16, name=f"T_c_u{u}", tag=f"T_c_u{u}")
        nc.scalar.activation(T_c, cst.c[u][wp], AF.Tanh)
        nc.vector.tensor_mul(cst.h[u][wp], S[3], T_c)
        if s + 1 < n_stage:
            nc.vector.tensor_copy(cst.xh[u][wp][0:64, :], cst.h[u][wp][0:64, :])

    S0 = [None] * 4
    S1 = [None] * 4
    for j in MMORD:
        S0[j] = gate_act(0, j)
    dve_cchain(0, S0)
    finish_unit(0, S0)  # T_c_u0 immediately after u0's c': starts the
    for j in MMORD:     # h0->copy->matmul tail a full ACT slot earlier
        S1[j] = gate_act(1, j)
    dve_cchain(1, S1)
    finish_unit(1, S1)

    if s == n_stage - 1:
        for u in range(NSUB):
            P_fc = gpool.tile([1, BSUB], F32, name=f"P_fc_u{u}", tag=f"P_u{u}")
            nc.tensor.matmul(P_fc, cst.wfc, cst.h[u][wp], start=True, stop=True)
            S_out = spool.tile([1, BSUB], F32, name=f"S_out_u{u}", tag=f"S_out_u{u}")
            nc.scalar.activation(S_out, P_fc, AF.Sigmoid, bias=cst.bfc[0:1, 0:1])
            nc.sync.dma_start(cst.out_d[u * BSUB : (u + 1) * BSUB, :], S_out)


def _build_module(t_steps=T_FULL):
    n_stage = t_steps + 1
    n_ctx0 = 128
    nc = bacc.Bacc("TRN2", target_bir_lowering=False, debug=False, enable_asserts=False)
    x2_d = nc.dram_tensor("x2", [BC, XCOLS], F32, kind="ExternalInput").ap()
    w0_d = nc.dram_tensor("w0", [72, 256], F8, kind="ExternalInput").ap()
    w1_d = nc.dram_tensor("w1", [128, 256], F8, kind="ExternalInput").ap()
    wfc_d = nc.dram_tensor("wfc", [128, 1], BF16, kind="ExternalInput").ap()
    bias_d = nc.dram_tensor("biases", [128, 8], F32, kind="ExternalInput").ap()
    bfc_d = nc.dram_tensor("bfc", [1, 1], F32, kind="ExternalInput").ap()
    id_d = nc.dram_tensor("ident", [128, 128], F32, kind="ExternalInput").ap()
    out_d = nc.dram_tensor("out", [BC, 1], F32, kind="ExternalOutput").ap()

    cst = _Consts()
    cst.w0 = nc.alloc_sbuf_tensor("w0_sb", [72, 256], F8).ap()
    cst.w1 = nc.alloc_sbuf_tensor("w1_sb", [128, 256], F8).ap()
    cst.wfc = nc.alloc_sbuf_tensor("wfc_sb", [128, 1], BF16).ap()
    cst.biases = nc.alloc_sbuf_tensor("bias_sb", [128, 8], F32).ap()
    cst.bfc = nc.alloc_sbuf_tensor("bfc_sb", [1, 1], F32).ap()
    cst.ident = nc.alloc_sbuf_tensor("ident_sb", [128, 128], F32).ap()
    cst.xTones = nc.alloc_sbuf_tensor("xTones_sb", [126, NBLK * BC], BF16).ap()
    cst.xs = [
        nc.alloc_sbuf_tensor(f"xs{ch}", [128, XCOLS], F32).ap() for ch in range(8)
    ]
    cst.out_d = out_d
    cst.h = [
        [nc.alloc_sbuf_tensor(f"h_u{u}_p{p}", [128, BSUB], BF16).ap() for p in range(2)]
        for u in range(NSUB)
    ]
    cst.c = [
        [nc.alloc_sbuf_tensor(f"c_u{u}_p{p}", [128, BSUB], BF16).ap() for p in range(2)]
        for u in range(NSUB)
    ]
    cst.xh = [
        [nc.alloc_sbuf_tensor(f"xh_u{u}_p{p}", [72, BSUB], BF16).ap() for p in range(2)]
        for u in range(NSUB)
    ]

    EARLY = 2 * 126  # columns for the two upfront transpose blocks

    with tile.TileContext(nc) as tc:
        with tc.sbuf_pool(name="state0", bufs=2) as spool:
            nc.sync.dma_start(cst.ident, id_d)
            nc.sync.dma_start(cst.w0, w0_d)
            nc.sync.dma_start(cst.w1, w1_d)
            nc.sync.dma_start(cst.wfc, wfc_d)
            nc.sync.dma_start(cst.biases, bias_d)
            nc.sync.dma_start(cst.bfc, bfc_d)
            for ch in range(8):
                eng = nc.sync if ch % 2 == 0 else nc.scalar
                eng.dma_start(
                    cst.xs[ch][:, 0:EARLY], x2_d[ch * 128 : (ch + 1) * 128, 0:EARLY]
                )
            for ch in range(8):
                eng = nc.sync if ch % 2 == 0 else nc.scalar
                eng.dma_start(
                    cst.xs[ch][:, EARLY:XCOLS],
                    x2_d[ch * 128 : (ch + 1) * 128, EARLY:XCOLS],
                )
            for u in range(NSUB):
                nc.gpsimd.memset(cst.h[u][0], 0.0)
                nc.gpsimd.memset(cst.c[u][0], 0.0)
                nc.gpsimd.memset(cst.xh[u][0][0:64, :], 0.0)

            with tc.psum_pool(name="pg0", bufs=1) as gpool:
                _emit_block_transpose(nc, gpool, cst, 0, "P_u0")
                _emit_block_transpose(nc, gpool, cst, 1, "P_u1")
                for u in range(NSUB):
                    nc.sync.dma_start(
                        cst.xh[u][0][64:72, :],
                        cst.xTones[0:8, u * BSUB : (u + 1) * BSUB],
                    )
                for s in range(n_ctx0):
                    _emit_stage(nc, spool, gpool, cst, s, n_stage)

    with tile.TileContext(nc) as tc:
        with tc.sbuf_pool(name="state1", bufs=2) as spool:
            with tc.psum_pool(name="pg1", bufs=1) as gpool:
                for s in range(n_ctx0, n_stage):
                    _emit_stage(nc, spool, gpool, cst, s, n_stage)

    nc.compile()
    return nc


def _prep_weights(Wih0, Whh0, bih0, bhh0, Wih1, Whh1, bih1, bhh1, Wfc, bfc):
    bf = ml_dtypes.bfloat16
    b0 = (bih0 + bhh0).astype(np.float32)
    b1 = (bih1 + bhh1).astype(np.float32)
    w0 = np.zeros((72, 256), np.float32)
    w1 = np.zeros((128, 256), np.float32)
    biases = np.zeros((128, 8), np.float32)
    for j, g in enumerate(GORDER):
        r = slice(g * 64, (g + 1) * 64)
        w0[0:64, j * 64 : (j + 1) * 64] = Whh0[r].T
        w0[64:72, j * 64 : (j + 1) * 64] = Wih0[r].T
        w1[0:64, j * 64 : (j + 1) * 64] = Wih1[r].T
        w1[64:128, j * 64 : (j + 1) * 64] = Whh1[r].T
        biases[0:64, j] = b0[r]
        biases[64:128, j] = b1[r]
        biases[0:64, 4 + j] = b0[r]  # stage-0 columns: l1 half stays 0
    wfc = np.concatenate(
        [np.zeros((64, 1), np.float32), Wfc.reshape(1, 64).T], axis=0
    )
    ident = np.eye(128, dtype=np.float32)
    f8 = ml_dtypes.float8_e4m3
    return (
        w0.astype(f8),
        w1.astype(f8),
        wfc.astype(bf),
        biases,
        np.array([[bfc[0]]], dtype=np.float32),
        ident,
    )


def _prep_x2(x):
    B = x.shape[0]
    x2 = np.zeros((B, NBLK * TPB, SLOT), np.float32)
    x2[:, :T_FULL, 0:D] = x
    x2[:, :, D] = 1.0
    return np.ascontiguousarray(x2.reshape(B, XCOLS))


_MODULE_CACHE = {}


def _get_module(t_steps=T_FULL):
    if t_steps not in _MODULE_CACHE:
        _MODULE_CACHE[t_steps] = _build_module(t_steps)
    return _MODULE_CACHE[t_steps]


def _run(inputs, trace=False, **spmd_kwargs):
    x = np.asarray(inputs["x"], np.float32)
    w0, w1, wfc, biases, bfc, ident = _prep_weights(
        np.asarray(inputs["Wih0"], np.float32),
        np.asarray(inputs["Whh0"], np.float32),
        np.asarray(inputs["bih0"], np.float32),
        np.asarray(inputs["bhh0"], np.float32),
        np.asarray(inputs["Wih1"], np.float32),
        np.asarray(inputs["Whh1"], np.float32),
        np.asarray(inputs["bih1"], np.float32),
        np.asarray(inputs["bhh1"], np.float32),
        np.asarray(inputs["Wfc"], np.float32),
        np.asarray(inputs["bfc"], np.float32),
    )
    x2 = _prep_x2(x)
    nc = _get_module(T_FULL)
    in_maps = []
    for c in range(N_CORES):
        in_maps.append(
            {
                "x2": x2[c * BC : (c + 1) * BC],
                "w0": w0,
                "w1": w1,
                "wfc": wfc,
                "biases": biases,
                "bfc": bfc,
                "ident": ident,
            }
        )
    res = run_bass_kernel_spmd(
        nc, in_maps, core_ids=list(range(N_CORES)), trace=trace, **spmd_kwargs
    )
    out = np.concatenate(
        [res.results[c]["out"] for c in range(N_CORES)], axis=0
    ).astype(np.float32)
    return out, res


def kernel(**inputs):
    out, _ = _run(inputs, trace=False)
    return out
